# revision 52
# baseline (speedup 1.0000x reference)
"""
Trainium2 Bass kernel for DynamicGraphAttention
(softmax(Hn Wq^T (Hn Wk^T)^T / sqrt(D) + eta*logit(clip(A)) masked)).

Shapes (hardcoded):
  Hn     [16, 2048, 256] f32
  A_stat [2048, 2048]    f32
  M_mask [2048, 2048]    int32
  Wq, Wk [256, 256]      f32
  out    [16, 2048, 2048] f32

Sharding across 8 NeuronCores: 2 batch-groups x 4 seq(query)-groups.
Core c handles batches of group bg = c // 4 and query rows
[qg*512:(qg+1)*512] (qg = c % 4). The mask is packed into the sign of
A on the host (am = m ? a : -1), Hn ships pre-transposed fp16.

Device algorithm (per core):
  G    = (Wq^T Wk) / sqrt(D)  fp32 matmul -> fp16   [256,256] (PE)
  btab = ln(clip(a)*ge + tiny) - ln(1 - clip(a)*ge)  fp16     (DVE+ACT)
         (ge = mask recovered from the sign of am; masked -> -69)
  VT   = G^T HqT  per batch, fp16                   [256,512] (PE)
  S    = VT.T @ HnT + I.btab  (fp16 matmuls) PSUM f32         (PE)
  p1   = exp(S) -> SBUF bf16, accum rowsum rs                 (ACT)
  out  = p1 * (1/rs)  bf16 -> DRAM                            (DVE)

Emission order = per-engine execution order, so bias prep / VT builds
are interleaved between the first sweep's tiles to overlap the ramp.
Output is bf16 on device, upcast to f32 on host.
"""

import math

import numpy as np

import concourse.bass as bass
import concourse.bacc as bacc
import concourse.tile as tile
from concourse import mybir
from concourse import bass_utils

F32 = mybir.dt.float32
F32R = mybir.dt.float32r
BF16 = mybir.dt.bfloat16
FP16 = mybir.dt.float16

B_FULL = 16
N = 2048
D = 256
NBG = 2   # batch groups
NQG = 4   # seq (query-row) groups
NB = B_FULL // NBG        # batches per core = 8
NQ = N // NQG             # query rows per core = 512
NQT = NQ // 128           # q tiles per core = 4
EPS = 1e-3
SCALE = 1.0 / math.sqrt(float(D))  # 1/16

_CACHE = {}


def _patch_act_tables():
    # Prefer the activation-table set that holds BOTH Ln and Exp so the
    # scalar engine never reloads tables between bias-prep logs and
    # softmax exps.
    from concourse import hw_specs as _hw
    if getattr(_hw, "_combined_first", False):
        return
    _orig = _hw.get_activation_tables

    def _patched(module_arch):
        tabs = _orig(module_arch)
        pref = "natural_log_exp_and_others"
        if pref in tabs:
            both = {mybir.ActivationFunctionType.Ln,
                    mybir.ActivationFunctionType.Exp,
                    mybir.ActivationFunctionType.Copy}
            tabs = {
                k: (v if k == pref else (v - both))
                for k, v in tabs.items()
            }
        return tabs

    _hw.get_activation_tables = _patched
    import concourse.bacc as _bacc_mod
    _bacc_mod.get_activation_tables = _patched
    _hw._combined_first = True


def _build():
    _patch_act_tables()
    nc = bacc.Bacc("TRN2", debug=False, enable_asserts=False)

    hnt_d = nc.dram_tensor("hnt", [NB, D, N], FP16, kind="ExternalInput").ap()
    hqt_d = nc.dram_tensor("hqt", [NB, D, NQ], FP16, kind="ExternalInput").ap()
    am_d = nc.dram_tensor("am", [NQ, N], F32, kind="ExternalInput").ap()
    wq_d = nc.dram_tensor("wq", [D, D], F32, kind="ExternalInput").ap()
    wk_d = nc.dram_tensor("wk", [D, D], F32, kind="ExternalInput").ap()
    idb_d = nc.dram_tensor("idb", [128, 128], FP16, kind="ExternalInput").ap()
    o_d = nc.dram_tensor("o", [NB, NQ, N], BF16, kind="ExternalOutput").ap()

    with tile.TileContext(nc) as tc:
        with (
            tc.tile_pool(name="consts", bufs=1) as consts,
            tc.tile_pool(name="amp", bufs=2) as amp,
            tc.tile_pool(name="prep", bufs=2) as prep,
            tc.tile_pool(name="bpool", bufs=1) as bpool,
            tc.tile_pool(name="hntp", bufs=8) as hntp,
            tc.tile_pool(name="hqtp", bufs=16) as hqtp,
            tc.tile_pool(name="vtp", bufs=16) as vtp,
            tc.tile_pool(name="pp", bufs=2) as pp,
            tc.tile_pool(name="op", bufs=2) as op_pool,
            tc.tile_pool(name="rsp", bufs=8) as rsp,
            tc.tile_pool(name="ps_s", bufs=2, space="PSUM") as ps_s,
        ):
            # ---- tiny const + ACT table preload (Ln/Exp/Copy set) ----
            tinyc = consts.tile([128, 1], F32, tag="tiny")
            nc.vector.memset(tinyc, 1e-30)
            warm = consts.tile([128, 1], F32, tag="warm")
            nc.scalar.activation(
                out=warm, in_=tinyc,
                func=mybir.ActivationFunctionType.Ln, bias=1.0, scale=1.0,
            )

            # ---- constants ----
            wq_sb = consts.tile([128, 2, D], F32, tag="wq")
            nc.sync.dma_start(out=wq_sb, in_=wq_d.rearrange("(c p) d -> p c d", p=128))
            wk_sb = consts.tile([128, 2, D], F32, tag="wk")
            nc.sync.dma_start(out=wk_sb, in_=wk_d.rearrange("(c p) d -> p c d", p=128))


            idb = consts.tile([128, 128], FP16, tag="idb")
            nc.sync.dma_start(out=idb, in_=idb_d)

            # am tiles (gpsimd queue so they land early, independent of
            # the sync ring that streams hqt/hnt); am[3] is emitted later
            # so it never blocks the SWDGE ring head on a buffer WAR.
            am_tiles = {}

            def emit_am(t):
                am_t = amp.tile([128, N], F32, tag="am", name=f"am{t}")
                nc.gpsimd.dma_start(out=am_t, in_=am_d[t * 128:(t + 1) * 128, :])
                am_tiles[t] = am_t

            for t in range(2):
                emit_am(t)

            def emit_hnt(b):
                hnt = []
                for i in range(2):
                    h_i = hntp.tile([128, N], FP16, tag="hnt", name=f"hnt{b}_{i}")
                    nc.sync.dma_start(
                        out=h_i, in_=hnt_d[b, i * 128:(i + 1) * 128, :]
                    )
                    hnt.append(h_i)
                return hnt

            hnts = {}
            hnts[0] = emit_hnt(0)

            # hqt loads (all upfront; VTs are built upfront)
            hqts = []
            for b in range(NB):
                hq = []
                for i in range(2):
                    hq_i = hqtp.tile([128, NQ], FP16, tag="hqt", name=f"hqt{b}_{i}")
                    nc.sync.dma_start(
                        out=hq_i, in_=hqt_d[b, i * 128:(i + 1) * 128, :]
                    )
                    hq.append(hq_i)
                hqts.append(hq)

            hnts[1] = emit_hnt(1)

            # ---- G = (Wq^T Wk) * SCALE : [256,256] as 2 tiles ----
            g = []
            for i in range(2):
                gp = ps_s.tile([128, N], F32, tag="s", name=f"gp{i}")
                for e in range(2):
                    nc.tensor.matmul(
                        gp[:, :D],
                        lhsT=wq_sb[:, e, i * 128:(i + 1) * 128],
                        rhs=wk_sb[:, e, :],
                        start=(e == 0),
                        stop=(e == 1),
                    )
                g_i = consts.tile([128, D], FP16, tag=f"g{i}", name=f"g{i}")
                nc.scalar.mul(out=g_i, in_=gp[:, :D], mul=SCALE)
                g.append(g_i)

            # ---- bias table prep (all DVE + ACT; gpsimd is too slow and
            # steals the DVE SBUF port) ----
            # btab[t] = ln(acm + tiny) - ln(1 - acm)   (f32r)
            # where ge = (am >= 0), acm = clip(am, eps, 1-eps) * ge
            # (masked entries: acm = 0 -> btab = ln(tiny) ~ -69)
            btab = []
            for t in range(NQT):
                btab.append(bpool.tile([128, N], FP16, tag=f"bt{t}", name=f"bt{t}"))

            def emit_bprep(t, split=1):
                am_t = am_tiles[t]
                w = N // split
                for h in range(split):
                    sl = slice(h * w, (h + 1) * w)
                    ge = prep.tile([128, w], FP16, tag="ge", name=f"ge{t}{h}")
                    nc.vector.tensor_scalar(
                        out=ge, in0=am_t[:, sl], scalar1=0.0, scalar2=None,
                        op0=mybir.AluOpType.is_ge,
                    )
                    ac = prep.tile([128, w], F32, tag="ac", name=f"ac{t}{h}")
                    nc.vector.tensor_scalar(
                        out=ac, in0=am_t[:, sl], scalar1=float(EPS),
                        scalar2=float(1.0 - EPS),
                        op0=mybir.AluOpType.max, op1=mybir.AluOpType.min,
                    )
                    acm = prep.tile([128, w], F32, tag="acm", name=f"acm{t}{h}")
                    nc.vector.tensor_mul(acm, ac, ge)
                    la = prep.tile([128, w], F32, tag="la", name=f"la{t}{h}")
                    nc.scalar.activation(
                        out=la, in_=acm, func=mybir.ActivationFunctionType.Ln,
                        bias=tinyc, scale=1.0,
                    )
                    l1a = prep.tile([128, w], F32, tag="l1a", name=f"l1a{t}{h}")
                    nc.scalar.activation(
                        out=l1a, in_=acm, func=mybir.ActivationFunctionType.Ln,
                        bias=1.0, scale=-1.0,
                    )
                    nc.vector.tensor_sub(btab[t][:, sl], la, l1a)

            # ---- VT builder ----
            def emit_vt(b):
                vt = []
                for j in range(2):
                    vt_j = vtp.tile([128, NQ], FP16, tag="vt", name=f"vt{b}_{j}")
                    vp = ps_s.tile(
                        [128, N], F32, tag="s", name=f"vp{b}{j}"
                    )[:, :NQ]
                    for i in range(2):
                        nc.tensor.matmul(
                            vp,
                            lhsT=g[i][:, j * 128:(j + 1) * 128],
                            rhs=hqts[b][i],
                            start=(i == 0),
                            stop=(i == 1),
                        )
                    nc.vector.tensor_copy(out=vt_j, in_=vp)
                    vt.append(vt_j)
                return vt

            vts = {}

            def emit_qtile(b, qt):
                qsl = slice(qt * 128, (qt + 1) * 128)
                vt, hnt = vts[b], hnts[b]
                s_ps = ps_s.tile([128, N], F32, tag="s", name=f"s{b}{qt}")
                for j in range(2):
                    for c in range(4):
                        csl = slice(c * 512, (c + 1) * 512)
                        nc.tensor.matmul(
                            s_ps[:, csl],
                            lhsT=vt[j][:, qsl],
                            rhs=hnt[j][:, csl],
                            start=(j == 0),
                            stop=False,
                        )
                for c in range(4):
                    csl = slice(c * 512, (c + 1) * 512)
                    nc.tensor.matmul(
                        s_ps[:, csl], lhsT=idb, rhs=btab[qt][:, csl],
                        start=False, stop=True,
                    )
                rs = rsp.tile([128, 1], F32, tag="rs", name=f"rs{b}{qt}")
                p1 = pp.tile([128, N], BF16, tag="p", name=f"p{b}{qt}")
                nc.scalar.activation(
                    out=p1, in_=s_ps,
                    func=mybir.ActivationFunctionType.Exp,
                    accum_out=rs,
                )
                rinv = rsp.tile([128, 1], F32, tag="rinv", name=f"ri{b}{qt}")
                nc.vector.reciprocal(out=rinv, in_=rs)
                out_t = op_pool.tile([128, N], BF16, tag="o", name=f"o{b}{qt}")
                nc.vector.tensor_scalar(
                    out=out_t, in0=p1, scalar1=rinv, scalar2=None,
                    op0=mybir.AluOpType.mult,
                )
                nc.gpsimd.dma_start(out=o_d[b, qsl, :], in_=out_t)

            # ---- sweep A: qt-major over batches 0..1. Per-engine
            # execution follows emission order, so bias prep and VT
            # builds are interleaved BETWEEN tiles to overlap the ramp.
            emit_bprep(0)
            vts[0] = emit_vt(0)
            vts[1] = emit_vt(1)
            for qt in range(NQT):
                if qt < NQT - 1:
                    emit_bprep(qt + 1)
                    vts[2 * qt + 2] = emit_vt(2 * qt + 2)
                    vts[2 * qt + 3] = emit_vt(2 * qt + 3)
                if qt == 0:
                    emit_am(2)
                if qt == 1:
                    emit_am(3)
                    hnts[2] = emit_hnt(2)
                if qt == 2:
                    hnts[3] = emit_hnt(3)
                for b in range(2):
                    emit_qtile(b, qt)

            # ---- sweep B: b-major over batches 2..7 ----
            for b in range(2, NB):
                if b + 2 < NB:
                    hnts[b + 2] = emit_hnt(b + 2)
                for qt in range(NQT):
                    emit_qtile(b, qt)
    nc.compile()
    return nc


def _get_nc():
    if "nc" not in _CACHE:
        _CACHE["nc"] = _build()
    return _CACHE["nc"]


def make_in_maps(Hn, A_stat, M_mask, Wq, Wk):
    Hn = np.ascontiguousarray(np.asarray(Hn, dtype=np.float32))
    A_stat = np.ascontiguousarray(np.asarray(A_stat, dtype=np.float32))
    M_mask = np.asarray(M_mask)
    Wq = np.ascontiguousarray(np.asarray(Wq, dtype=np.float32))
    Wk = np.ascontiguousarray(np.asarray(Wk, dtype=np.float32))
    assert Hn.shape == (B_FULL, N, D)

    # mask packed into the sign: masked entries become -1.0
    am_full = np.where(M_mask != 0, A_stat, np.float32(-1.0)).astype(np.float32)

    # [16, 256, 2048] transposed-node layout, fp16
    hnt_full = np.ascontiguousarray(Hn.astype(np.float16).transpose(0, 2, 1))

    in_maps = []
    for c in range(8):
        bg, qg = c // NQG, c % NQG
        bsl = slice(bg * NB, (bg + 1) * NB)
        qsl = slice(qg * NQ, (qg + 1) * NQ)
        in_maps.append({
            "hnt": hnt_full[bsl],
            "hqt": np.ascontiguousarray(hnt_full[bsl][:, :, qsl]),
            "am": np.ascontiguousarray(am_full[qsl]),
            "wq": Wq,
            "wk": Wk,
            "idb": np.eye(128, dtype=np.float16),
        })
    return in_maps


def assemble(results):
    out = np.empty((B_FULL, N, N), dtype=np.float32)
    for c in range(8):
        bg, qg = c // NQG, c % NQG
        o = results[c]["o"]
        out[bg * NB:(bg + 1) * NB, qg * NQ:(qg + 1) * NQ, :] = (
            np.asarray(o).astype(np.float32)
        )
    return out


def kernel(Hn, A_stat, M_mask, Wq, Wk):
    in_maps = make_in_maps(Hn, A_stat, M_mask, Wq, Wk)
    nc = _get_nc()
    res = bass_utils.run_bass_kernel_spmd(nc, in_maps, core_ids=list(range(8)))
    return assemble(res.results)


if __name__ == "__main__":
    rng = np.random.default_rng(0)
    inputs = {
        "Hn": rng.standard_normal((B_FULL, N, D), dtype=np.float32),
        "A_stat": rng.random((N, N), dtype=np.float32),
        "M_mask": rng.integers(0, 2, size=(N, N), dtype=np.int32),
        "Wq": rng.standard_normal((D, D), dtype=np.float32) / 16,
        "Wk": rng.standard_normal((D, D), dtype=np.float32) / 16,
    }
    out = kernel(**inputs)
    print(out.shape, out.dtype, out.sum())
